# revision 7
# baseline (speedup 1.0000x reference)
"""MultiHeadEMA (MEGA bidirectional EMA + residual + SiLU) on 8 Trainium2 cores.

Strategy
--------
Per channel d (E=1024, B=4, L=4096):
    y[n] = silu( sum_{m<=n} x[m] k1[d, n-m] + sum_{m>n} x[m] k2[d, m-n-1]
                 + omega[d] x[n] )
with k1/k2 16-term geometric mixtures, q = 1 - sigmoid(a)*sigmoid(d) <= 0.865.
q^32 tail: worst-channel L1 1.4e-2 << 2e-2 * scale(16.9), so the length-2L FFT
conv reduces to a +-T=32-tap banded conv done by overlap-save with DFT F=256,
hop C=192 (22 windows).  E sharded 8 ways (128 ch/core, FREE = B*128 = 512).

Matmul cost on TRN2 is free-dim cycles (~216 ns at N=512 fp16) PLUS ~100 ns
whenever a partial-partition (row_grp/col_grp) matmul breaks LDWEIGHTS
pipelining -- so every matmul here is full 128x128 config: the odd-window
forward chunks and the half-block inverse ranges use ZERO-PADDED weight
matrices instead of partial partition ranges (zero rows/cols stream for
free).  194 matmuls total vs 300 for the F=512/T=64 baseline.

Freq packing (256-pt real DFT -> 256 real rows in 2 PSUM banks):
    X0 rows f=0..127:  Re X[f]
    X1 row 0: Re X[128] (Nyquist); rows 1..127: Im X[f]
Pointwise complex multiply via 4 coefficient planes:
    Y0 = A0*X0 + B0*X1 ; Y1 = A1*X1 + B1*X0
omega residual folded into tap 0 host-side.

Windows are processed in PAIRS (w, w+1): each forward DFT writes its bank of
a 2-bank PSUM tile, and all elementwise ops run once per pair at FD=1024,
halving DVE/ACT per-op init overhead (DVE and GpSimd share an SBUF port, so
op COUNT is what matters).  Split output blocks accumulate two windows'
inverse matmuls in one PSUM bank (second window start=False).

Per-pair engine split:
    PE : fwd 8-12 + inv 8 matmuls (~220 ns each)
    DVE: t00=A0*X0p, t11=B1*X0p from PSUM (~1.2 us) + 2 adds (~0.6 us)
    ACT: X1p copy PSUM->SBUF fp16 (~1.15 us) + 3 silu (~0.7 us each)
    GPS: t01=B0*x1s (+t10=A1*x1s on odd pairs; DVE takes it on even pairs)
fwd(p+1) is emitted before inv(p) so the PE queue stays fed while the
pointwise chain of pair p completes.
"""

import math
import numpy as np
from contextlib import ExitStack

import concourse.bass as bass
import concourse.tile as tile
from concourse import bacc, mybir
from concourse.bass_utils import run_bass_kernel_spmd

L, B, E, NDIM = 4096, 4, 1024, 16
N_CORES = 8
ESH = E // N_CORES            # 128 channels per core
F, T, C = 256, 32, 192        # DFT length, one-sided taps, hop
NW = (L + C - 1) // C         # 22 windows (last covers 64 outputs)
NP = (NW + 1) // 2            # 11 window pairs
FREE = B * ESH                # 512 free elements (b, chan)
NXT = 33                      # x tiles: rows [0, 4224), x at [T, T+L)
NBLK = L // 128               # 32 output blocks

F16 = mybir.dt.float16
F32 = mybir.dt.float32

LAST_RESULTS = None           # BassKernelResults of the most recent run
_CACHE: dict = {}


def _build_nc():
    nc = bacc.Bacc("TRN2", target_bir_lowering=False, debug=False,
                   num_devices=N_CORES)
    xs = nc.dram_tensor("xs", [NXT * 128, B, ESH], F16, kind="ExternalInput").ap()
    wf = nc.dram_tensor("wf", [128, 2, 2, 128], F16, kind="ExternalInput").ap()
    wo = nc.dram_tensor("wo", [128, 2, 3, 128], F16, kind="ExternalInput").ap()
    vi = nc.dram_tensor("vi", [128, 2, 4, 128], F16, kind="ExternalInput").ap()
    kco = nc.dram_tensor("kco", [128, 4, 2, FREE], F16, kind="ExternalInput").ap()
    out = nc.dram_tensor("out", [L, B, ESH], F16, kind="ExternalOutput").ap()

    with ExitStack() as ctx:
        tc = ctx.enter_context(tile.TileContext(nc))
        cpool = ctx.enter_context(tc.tile_pool(name="const", bufs=1))
        ppool = ctx.enter_context(tc.tile_pool(name="pw", bufs=2))
        opool = ctx.enter_context(tc.tile_pool(name="outp", bufs=3))
        ps_f = ctx.enter_context(tc.tile_pool(name="psf", bufs=1, space="PSUM"))
        ps_i = ctx.enter_context(tc.tile_pool(name="psi", bufs=1, space="PSUM"))

        # DMA order = first-use order; spread across Sync + Scalar HWDGE
        # queues so issue overlaps.
        wf_t = cpool.tile([128, 2, 2, 128], F16)
        nc.scalar.dma_start(wf_t[:], wf)
        x_all = cpool.tile([128, NXT, FREE], F16)
        xr = xs.rearrange("(t p) b c -> p t (b c)", p=128)
        nc.sync.dma_start(x_all[:, 0:4, :], xr[:, 0:4, :])
        wo_t = cpool.tile([128, 2, 3, 128], F16)
        nc.scalar.dma_start(wo_t[:], wo)
        k_t = cpool.tile([128, 4, 2, FREE], F16)
        nc.scalar.dma_start(k_t[:], kco)
        vi_t = cpool.tile([128, 2, 4, 128], F16)
        nc.scalar.dma_start(vi_t[:], vi)
        nc.sync.dma_start(x_all[:, 4:10, :], xr[:, 4:10, :])
        for t0 in range(10, NXT, 6):
            t1 = min(t0 + 6, NXT)
            nc.sync.dma_start(x_all[:, t0:t1, :], xr[:, t0:t1, :])

        def fwd(w, xh):
            """forward 256-pt real DFT of window w -> bank w%2 of X0/X1 pair"""
            if w % 2 == 0:
                a = 3 * w // 2
                chunks = [(x_all[:, a, :], wf_t, 0),
                          (x_all[:, a + 1, :], wf_t, 1)]
                wt = wf_t
            else:
                a = (3 * w - 1) // 2
                chunks = [(x_all[:, a, :], wo_t, 1),
                          (x_all[:, a + 1, :], wo_t, 0)]
                if a + 2 < NXT:
                    chunks.append((x_all[:, a + 2, :], wo_t, 2))
            n = len(chunks)
            for b in range(2):
                for k, (rh, wt, ki) in enumerate(chunks):
                    nc.tensor.matmul(xh[b][:, w % 2, :], wt[:, b, ki, :], rh,
                                     start=(k == 0), stop=(k == n - 1))

        blk_tiles = {}

        def get_blk(bi):
            if bi not in blk_tiles:
                blk_tiles[bi] = ps_i.tile([128, FREE], F32, tag=f"yi{bi % 3}",
                                          name=f"yi{bi}")
            return blk_tiles[bi]

        def silu_store(bi):
            yi = blk_tiles.pop(bi)
            o_sb = opool.tile([128, FREE], F16, tag=f"o{bi % 3}", name=f"o{bi}")
            nc.scalar.activation(o_sb[:], yi[:],
                                 mybir.ActivationFunctionType.Silu)
            nc.sync.dma_start(
                out[128 * bi: 128 * (bi + 1), :, :].rearrange("p b c -> p (b c)"),
                o_sb[:])

        def pointwise(p, xh):
            """FD=1024 elementwise over the window pair (2*p, 2*p+1)"""
            X0, X1 = xh
            x1s = ppool.tile([128, 2, FREE], F16, tag="x1s", name=f"x1s_{p}")
            nc.scalar.copy(x1s[:], X1[:])
            t00 = ppool.tile([128, 2, FREE], F16, tag="t00", name=f"t00_{p}")
            nc.vector.tensor_mul(t00[:], X0[:], k_t[:, 0, :, :])
            t11 = ppool.tile([128, 2, FREE], F16, tag="t11", name=f"t11_{p}")
            nc.vector.tensor_mul(t11[:], X0[:], k_t[:, 3, :, :])
            t01 = ppool.tile([128, 2, FREE], F16, tag="t01", name=f"t01_{p}")
            nc.gpsimd.tensor_mul(t01[:], x1s[:], k_t[:, 1, :, :])
            t10 = ppool.tile([128, 2, FREE], F16, tag="t10", name=f"t10_{p}")
            if p % 2 == 0:
                nc.vector.tensor_mul(t10[:], x1s[:], k_t[:, 2, :, :])
            else:
                nc.gpsimd.tensor_mul(t10[:], x1s[:], k_t[:, 2, :, :])
            y0 = ppool.tile([128, 2, FREE], F16, tag="y0", name=f"y0_{p}")
            nc.vector.tensor_add(y0[:], t00[:], t01[:])
            y1 = ppool.tile([128, 2, FREE], F16, tag="y1", name=f"y1_{p}")
            nc.vector.tensor_add(y1[:], t10[:], t11[:])
            return y0, y1

        def inv(w, y0, y1):
            """inverse DFT + silu + store for window w (rhs = bank w%2)"""
            # ranges: (block, vseg, first_writer); V segs are zero-padded to
            # 128 cols so every matmul writes the full partition range.
            if w % 2 == 0:
                ranges = [(3 * w // 2, 0, True),
                          (3 * w // 2 + 1, 1, True)]
            else:
                ranges = [((3 * w - 1) // 2, 3, False),
                          ((3 * w + 1) // 2, 2, True)]
            ranges = [r for r in ranges if r[0] < NBLK]
            for bi, seg, first in ranges:
                yi = get_blk(bi)
                nc.tensor.matmul(yi[:], vi_t[:, 0, seg, :], y0[:, w % 2, :],
                                 start=first, stop=False, skip_group_check=True)
                nc.tensor.matmul(yi[:], vi_t[:, 1, seg, :], y1[:, w % 2, :],
                                 start=False, stop=True, skip_group_check=True)
            if w % 2 == 0:
                silu_store(3 * w // 2)
            else:
                silu_store((3 * w - 1) // 2)
                if (3 * w + 1) // 2 < NBLK:
                    silu_store((3 * w + 1) // 2)

        # PE pre-warm: dummy matmuls keep the HAM activity monitor busy while
        # the first x tiles stream in, so real matmuls start near 2.4 GHz.
        warm = ps_i.tile([128, FREE], F32, tag="yi0", name="warm")
        for r in range(16):
            nc.tensor.matmul(warm[:, 0:256], wf_t[:, 0, 0, :],
                             wf_t[:, 0, :, :], start=(r == 0), stop=(r == 15))

        # pair-skewed pipeline: fwd(pair p+1) enters the PE queue before
        # inv(pair p), which waits on pair p's pointwise chain.
        def fwd_pair(p, xh):
            fwd(2 * p, xh)
            if 2 * p + 1 < NW:
                fwd(2 * p + 1, xh)

        xh_cur = [ps_f.tile([128, 2, FREE], F32, tag=f"x{b}", name=f"x{b}_0")
                  for b in range(2)]
        fwd_pair(0, xh_cur)
        for p in range(NP):
            y0, y1 = pointwise(p, xh_cur)
            if p + 1 < NP:
                xh_next = [ps_f.tile([128, 2, FREE], F32, tag=f"x{b}",
                                     name=f"x{b}_{p + 1}") for b in range(2)]
                fwd_pair(p + 1, xh_next)
            else:
                xh_next = None
            inv(2 * p, y0, y1)
            if 2 * p + 1 < NW:
                inv(2 * p + 1, y0, y1)
            xh_cur = xh_next
    nc.compile()
    return nc


def _host_prep(x, alpha, delta, beta, gamma, omega):
    """Fold EMA params into freq-domain coefficient planes + DFT matrices."""
    a = 1.0 / (1.0 + np.exp(-alpha.astype(np.float64)))
    d = 1.0 / (1.0 + np.exp(-delta.astype(np.float64)))
    q = 1.0 - a * d                               # (2E, 16, 1)
    w = (a * beta.astype(np.float64))[:, :, 0] * gamma.astype(np.float64)
    w *= math.sqrt(1.0 / NDIM)                    # (2E, 16)
    tau = np.arange(128)
    kern = (w[:, :, None] * q[:, :, 0:1] ** tau[None, None, :]).sum(1)  # (2E,128)
    k1, k2 = kern[:E], kern[E:]
    kc = np.zeros((E, F))
    kc[:, 0:128] = k1
    kc[:, F - 127:] = k2[:, :127][:, ::-1]        # slot F-i holds k2[i-1]
    kc[:, 0] += omega.astype(np.float64)          # residual == omega on tap 0
    Khat = np.fft.rfft(kc, axis=1)                # (E, 129)
    KRe, KIm = Khat.real, Khat.imag

    planes = np.zeros((4, 128, E))                # A0, B0, A1, B1
    planes[0] = KRe[:, 0:128].T
    planes[1, 1:] = -KIm[:, 1:128].T
    planes[2, 0] = KRe[:, 128]
    planes[2, 1:] = KRe[:, 1:128].T
    planes[3, 1:] = KIm[:, 1:128].T

    # forward DFT lhsT packs: W0 (Re rows), W1 (Nyquist + Im rows) [256, 128]
    t_ = np.arange(F)
    fr = np.arange(128)
    W0 = np.cos(2 * np.pi * np.outer(t_, fr) / F)
    W1 = np.empty((F, 128))
    W1[:, 0] = np.cos(np.pi * t_)
    W1[:, 1:] = -np.sin(2 * np.pi * np.outer(t_, fr[1:]) / F)
    Wb = np.stack([W0, W1], axis=0)               # (2, 256, 128)
    wf = np.empty((128, 2, 2, 128))               # even: chunk k = rows 128k+p
    for k in range(2):
        wf[:, :, k, :] = Wb[:, 128 * k:128 * (k + 1), :].transpose(1, 0, 2)
    # odd windows: full 128-part chunks with zero-padded edge weights
    wo = np.zeros((128, 2, 3, 128))
    wo[:, :, 0, :] = Wb[:, 64:192, :].transpose(1, 0, 2)      # mid: tile a+1
    wo[64:128, :, 1, :] = Wb[:, 0:64, :].transpose(1, 0, 2)   # lo: tile a
    wo[0:64, :, 2, :] = Wb[:, 192:256, :].transpose(1, 0, 2)  # hi: tile a+2

    # inverse lhsT: V0/V1 [128 freq, 192 outs], zero-padded per range
    jj = np.arange(C) + T
    c_f = np.where(fr == 0, 1.0, 2.0)
    V0 = c_f[:, None] * np.cos(2 * np.pi * np.outer(fr, jj) / F) / F
    V1 = np.empty((128, C))
    V1[0] = ((-1.0) ** jj) / F
    V1[1:] = -2 * np.sin(2 * np.pi * np.outer(fr[1:], jj) / F) / F
    Vb = np.stack([V0, V1], axis=0)               # (2, 128, 192)
    vi = np.zeros((128, 2, 4, 128))
    vi[:, :, 0, :] = Vb[:, :, 0:128].transpose(1, 0, 2)      # even range A
    vi[:, :, 1, 0:64] = Vb[:, :, 128:192].transpose(1, 0, 2)  # even range B
    vi[:, :, 2, :] = Vb[:, :, 64:192].transpose(1, 0, 2)     # odd range B
    vi[:, :, 3, 64:128] = Vb[:, :, 0:64].transpose(1, 0, 2)  # odd range A

    xpad = np.zeros((NXT * 128, B, E), np.float16)
    xpad[T:T + L] = x.astype(np.float16)

    wf16 = np.ascontiguousarray(wf.astype(np.float16))
    wo16 = np.ascontiguousarray(wo.astype(np.float16))
    vi16 = np.ascontiguousarray(vi.astype(np.float16))
    in_maps = []
    for core in range(N_CORES):
        sl = slice(core * ESH, (core + 1) * ESH)
        kc1 = np.broadcast_to(
            planes.reshape(4, 128, 1, 1, E)[:, :, :, :, sl],
            (4, 128, 2, B, ESH)).reshape(4, 128, 2, FREE).transpose(1, 0, 2, 3)
        in_maps.append({
            "xs": np.ascontiguousarray(xpad[:, :, sl]),
            "wf": wf16,
            "wo": wo16,
            "vi": vi16,
            "kco": np.ascontiguousarray(kc1.astype(np.float16)),
        })
    return in_maps


def kernel(x, alpha, delta, beta, gamma, omega):
    global LAST_RESULTS
    if "nc" not in _CACHE:
        _CACHE["nc"] = _build_nc()
    nc = _CACHE["nc"]
    in_maps = _host_prep(x, alpha, delta, beta, gamma, omega)
    res = run_bass_kernel_spmd(nc, in_maps, core_ids=list(range(N_CORES)))
    LAST_RESULTS = res
    out = np.concatenate([res.results[c]["out"] for c in range(N_CORES)], axis=2)
    return out.astype(np.float32)
